# revision 50
# baseline (speedup 1.0000x reference)
"""Trainium2 Bass kernel for nn_MixtureOfAgents.

Contract: kernel(**inputs) takes FULL unsharded inputs (numpy) and returns the
FULL output [4, 4096, 768] float32.

Strategy (v2):
  - Reference quirk: for each of TOP_K=2 steps, ONE scalar agent id
    (top_i[0, -1, k]) selects the expert pair used for ALL tokens.  The host
    computes the full per-token routing (agent softmax, top-2 renorm, expert
    pair softmax -> 4 gate rows g, plus agent one-hots) and slices the 4
    selected expert FFN blocks.  The device runs only the dense pipeline:
    transpose x, 4x (mm1/mm3 -> silu*mul -> mm2 -> combine), transpose out.
  - Data-parallel over tokens: 8 cores x 2048 tokens, weights replicated.
  - All matmul operands in bf16 (weights/x/hidden); PSUM accumulation fp32;
    combine arithmetic fp32.  Weights are pre-laid on host so each SBUF tile
    is one contiguous DMA.
  - mm1/mm3 stream two 512-token halves per stationary weight load.
"""

import numpy as np

# ---- problem constants (hardcoded; kernel.py must be self-contained) ----
N_CORES = 8
B, T, C = 4, 4096, 768
TOK = B * T              # 16384
TPC = TOK // N_CORES     # 2048 tokens per core
CT = C // 128            # 6 c-tiles
FFN = 2048
FT = FFN // 128          # 16 f-tiles per expert
A = 10                   # n_agents
EPA = 2                  # experts per agent
TOPK = 2
NBLK = TPC // 128        # 16 token-blocks
NTQ = TPC // 512         # 4 token-quarters

_CACHE = {}


def _build_module():
    import concourse.bass as bass
    import concourse.bacc as bacc
    import concourse.mybir as mybir
    import concourse.tile as tile
    from concourse.masks import make_identity
    from contextlib import ExitStack

    f32 = mybir.dt.float32
    bf16 = mybir.dt.bfloat16
    AF = mybir.ActivationFunctionType
    OP = mybir.AluOpType

    nc = bacc.Bacc(target_bir_lowering=False)
    xs = nc.dram_tensor("xs", [2, 128, CT, TPC // 2], bf16, kind="ExternalInput")
    w1t = nc.dram_tensor("w1t", [TOPK, EPA, FT, 128, CT, 128], bf16,
                         kind="ExternalInput")
    w3t = nc.dram_tensor("w3t", [TOPK, EPA, FT, 128, CT, 128], bf16,
                         kind="ExternalInput")
    w2t = nc.dram_tensor("w2t", [TOPK, EPA, CT, 128, FT, 128], bf16,
                         kind="ExternalInput")
    rolef = nc.dram_tensor("rolef", [TOPK, 128, CT, TPC], bf16,
                           kind="ExternalInput")
    g4 = nc.dram_tensor("g4", [TOPK * EPA, 128, TPC], bf16, kind="ExternalInput")
    out = nc.dram_tensor("out", [CT, 128, TPC], f32, kind="ExternalOutput")
    import os as _os
    _dbg = _os.environ.get("KERNEL_DEBUG_DUMPS") == "1"
    if _dbg:
        dbg_hid = nc.dram_tensor("dbg_hid", [128, FT, TPC], bf16,
                                 kind="ExternalOutput")
        dbg_acc = nc.dram_tensor("dbg_acc", [128, CT, TPC], f32,
                                 kind="ExternalOutput")

    with ExitStack() as ctx:
        tc = ctx.enter_context(tile.TileContext(nc))
        const = ctx.enter_context(tc.tile_pool(name="const", bufs=1))
        persist = ctx.enter_context(tc.tile_pool(name="persist", bufs=1))
        w13p = ctx.enter_context(tc.tile_pool(name="w13p", bufs=6))
        w2p = ctx.enter_context(tc.tile_pool(name="w2p", bufs=2))
        btp = ctx.enter_context(tc.tile_pool(name="btp", bufs=2))
        rp = ctx.enter_context(tc.tile_pool(name="rp", bufs=1))
        tmpp = ctx.enter_context(tc.tile_pool(name="tmpp", bufs=2))
        psH = ctx.enter_context(tc.tile_pool(name="psH", bufs=4, space="PSUM"))
        psY = ctx.enter_context(tc.tile_pool(name="psY", bufs=4, space="PSUM"))


        xT = persist.tile([128, CT, TPC], bf16, tag="xT", name="xT")
        hid = persist.tile([128, FT, TPC], bf16, tag="hid", name="hid")
        acc = persist.tile([128, CT, TPC], f32, tag="acc", name="acc")

        # ---- prologue: first-pass weight tiles go out ahead of the bulk
        # x DMA so the PE can start as soon as the first x half lands ----
        pre_w = {}
        for f in range(2):
            w1f = w13p.tile([128, CT, 128], bf16, tag="w13", name=f"w1f_0_0_{f}")
            nc.sync.dma_start(out=w1f, in_=w1t[0, 0, f, :, :, :])
            w3f = w13p.tile([128, CT, 128], bf16, tag="w13", name=f"w3f_0_0_{f}")
            nc.sync.dma_start(out=w3f, in_=w3t[0, 0, f, :, :, :])
            pre_w[f] = (w1f, w3f)
        # x pre-transposed [C-part, c-tile, tok], two contiguous halves
        for h in range(2):
            hs = slice(h * (TPC // 2), (h + 1) * (TPC // 2))
            nc.sync.dma_start(out=xT[:, :, hs], in_=xs[h, :, :, :])

        # ---- main: 4 expert passes ----
        for k in range(TOPK):
            rk = rp.tile([128, CT, TPC], bf16, tag="rk", name=f"rk_{k}")
            nc.sync.dma_start(out=rk, in_=rolef[k, :, :, :])
            for e in range(EPA):
                # mm1 + mm3 + silu*mul -> hid [FFN, TPC]
                for f in range(FT):
                    if k == 0 and e == 0 and f in pre_w:
                        w1f, w3f = pre_w[f]
                    else:
                        w1f = w13p.tile([128, CT, 128], bf16, tag="w13",
                                        name=f"w1f_{k}_{e}_{f}")
                        nc.sync.dma_start(out=w1f, in_=w1t[k, e, f, :, :, :])
                        w3f = w13p.tile([128, CT, 128], bf16, tag="w13",
                                        name=f"w3f_{k}_{e}_{f}")
                        nc.sync.dma_start(out=w3f, in_=w3t[k, e, f, :, :, :])
                    for half in range(2):
                        t0 = slice(half * 1024, half * 1024 + 512)
                        t1 = slice(half * 1024 + 512, half * 1024 + 1024)
                        ph1 = [psH.tile([128, 512], f32, tag="ps_h",
                                        name=f"ph1_{k}_{e}_{f}_{half}_{q}")
                               for q in range(2)]
                        ph3 = [psH.tile([128, 512], f32, tag="ps_h",
                                        name=f"ph3_{k}_{e}_{f}_{half}_{q}")
                               for q in range(2)]
                        # pair token-halves per stationary weight tile
                        for c in range(CT):
                            nc.tensor.matmul(ph1[0], w1f[:, c, :], xT[:, c, t0],
                                             start=(c == 0), stop=(c == CT - 1))
                            nc.tensor.matmul(ph1[1], w1f[:, c, :], xT[:, c, t1],
                                             start=(c == 0), stop=(c == CT - 1))
                        for c in range(CT):
                            nc.tensor.matmul(ph3[0], w3f[:, c, :], xT[:, c, t0],
                                             start=(c == 0), stop=(c == CT - 1))
                            nc.tensor.matmul(ph3[1], w3f[:, c, :], xT[:, c, t1],
                                             start=(c == 0), stop=(c == CT - 1))
                        for q, ts in ((0, t0), (1, t1)):
                            nc.scalar.activation(hid[:, f, ts], ph1[q], AF.Silu)
                            nc.vector.tensor_tensor(hid[:, f, ts], hid[:, f, ts],
                                                    ph3[q], op=OP.mult)

                if _dbg and k == 0 and e == 0:
                    nc.sync.dma_start(out=dbg_hid[:, :, :], in_=hid[:, :, :])
                # gates needed only from the combine stage on
                bt = btp.tile([128, TPC], bf16, tag="bt", name=f"bt_{k}_{e}")
                nc.sync.dma_start(out=bt, in_=g4[2 * k + e, :, :])
                # mm2 + combine into acc
                for c in range(CT):
                    w2f = w2p.tile([128, FT, 128], bf16, tag="w2",
                                   name=f"w2f_{k}_{e}_{c}")
                    nc.sync.dma_start(out=w2f, in_=w2t[k, e, c, :, :, :])
                    for tq in range(NTQ):
                        ts = slice(tq * 512, (tq + 1) * 512)
                        py = psY.tile([128, 512], f32, tag="ps_y",
                                      name=f"py_{k}_{e}_{c}_{tq}")
                        for f in range(FT):
                            nc.tensor.matmul(py, w2f[:, f, :], hid[:, f, ts],
                                             start=(f == 0), stop=(f == FT - 1))
                        t1_ = tmpp.tile([128, 512], f32, tag="t1",
                                        name=f"t1_{k}_{e}_{c}_{tq}")
                        nc.vector.tensor_tensor(t1_, bt[:, ts], py,
                                                op=OP.mult)
                        if k == 0 and e == 0:
                            nc.vector.scalar_tensor_tensor(
                                out=acc[:, c, ts], in0=rk[:, c, ts], scalar=1.0,
                                in1=t1_, op0=OP.add, op1=OP.mult)
                        else:
                            nc.vector.scalar_tensor_tensor(
                                out=t1_, in0=rk[:, c, ts], scalar=1.0,
                                in1=t1_, op0=OP.add, op1=OP.mult)
                            nc.vector.tensor_tensor(acc[:, c, ts], acc[:, c, ts],
                                                    t1_, op=OP.add)

                if _dbg and k == 0 and e == 0:
                    nc.sync.dma_start(out=dbg_acc[:, :, :], in_=acc[:, :, :])

        # ---- epilogue: store acc in [c-tile, C-part, tok] layout ----
        for c in range(CT):
            for h in range(2):
                hs = slice(h * (TPC // 2), (h + 1) * (TPC // 2))
                nc.sync.dma_start(out=out[c, :, hs], in_=acc[:, c, hs])

    nc.compile()
    return nc


def _get_nc():
    if "nc" not in _CACHE:
        _CACHE["nc"] = _build_module()
    return _CACHE["nc"]


def _enable_jax_compile_cache():
    try:
        import jax
        jax.config.update("jax_compilation_cache_dir", "/tmp/jax_kernel_cache")
        jax.config.update("jax_persistent_cache_min_compile_time_secs", 1.0)
    except Exception:
        pass


def _host_routing(xf, agent_gate_w, expert_gate_w):
    """Per-token gates exactly as the reference computes them (fp32)."""
    al = xf @ agent_gate_w.T                                    # [TOK, A]
    al = al - al.max(axis=1, keepdims=True)
    aw = np.exp(al)
    aw /= aw.sum(axis=1, keepdims=True)
    order = np.argsort(-aw, axis=1, kind="stable")              # [TOK, A]
    i_k = order[:, :TOPK]                                       # [TOK, 2]
    w_k = np.take_along_axis(aw, i_k, axis=1)                   # [TOK, 2]
    tw = w_k / (w_k.sum(axis=1, keepdims=True) + 1e-6)          # [TOK, 2]

    # scalar agent ids from token (b=0, t=T-1) -> flat row T-1
    sel = [int(i_k[T - 1, k]) * EPA for k in range(TOPK)]

    cols = [sel[0], sel[0] + 1, sel[1], sel[1] + 1]
    el = xf @ expert_gate_w[cols].T                             # [TOK, 4]
    g = np.empty((4, TOK), dtype=np.float32)
    for k in range(TOPK):
        pair = el[:, 2 * k:2 * k + 2]
        pair = pair - pair.max(axis=1, keepdims=True)
        ew = np.exp(pair)
        ew /= ew.sum(axis=1, keepdims=True)
        g[2 * k] = tw[:, k] * ew[:, 0]
        g[2 * k + 1] = tw[:, k] * ew[:, 1]

    onehot = np.zeros((TOPK, A, TOK), dtype=np.float32)
    for k in range(TOPK):
        onehot[k, i_k[:, k], np.arange(TOK)] = 1.0
    return sel, g, onehot


def kernel(x, agent_gate_w, expert_gate_w, role_emb, w1, w2, w3,
           _trace=False, _dtype="f32r"):
    import ml_dtypes
    from concourse.bass_utils import run_bass_kernel_spmd

    _enable_jax_compile_cache()
    bf16 = ml_dtypes.bfloat16

    x = np.asarray(x, dtype=np.float32)
    agent_gate_w = np.asarray(agent_gate_w, dtype=np.float32)
    expert_gate_w = np.asarray(expert_gate_w, dtype=np.float32)
    role_emb = np.asarray(role_emb, dtype=np.float32)
    w1 = np.asarray(w1, dtype=np.float32)
    w2 = np.asarray(w2, dtype=np.float32)
    w3 = np.asarray(w3, dtype=np.float32)

    xf = np.ascontiguousarray(x.reshape(TOK, C))
    sel, g, onehot = _host_routing(xf, agent_gate_w, expert_gate_w)

    rows = [sel[0], sel[0] + 1, sel[1], sel[1] + 1]
    # w1/w3 tiles: [ke, f, p, c, j] with value w[e, f*128+j, c*128+p]
    w1sel = w1[rows].reshape(4, FT, 128, CT, 128).transpose(0, 1, 4, 3, 2)
    w3sel = w3[rows].reshape(4, FT, 128, CT, 128).transpose(0, 1, 4, 3, 2)
    # w2 tiles: [ke, c, p, f, j] with value w2[e, c*128+j, f*128+p]
    w2sel = w2[rows].reshape(4, CT, 128, FT, 128).transpose(0, 1, 4, 3, 2)

    w1tp = np.ascontiguousarray(
        w1sel.reshape(TOPK, EPA, FT, 128, CT, 128).astype(bf16))
    w3tp = np.ascontiguousarray(
        w3sel.reshape(TOPK, EPA, FT, 128, CT, 128).astype(bf16))
    w2tp = np.ascontiguousarray(
        w2sel.reshape(TOPK, EPA, CT, 128, FT, 128).astype(bf16))
    # gathered role factor 0.1*role_emb[idx_k] per token, [TOPK, TOK, CT, 128]
    role_s = 0.1 * role_emb
    idx = np.argmax(onehot, axis=1)                       # [TOPK, TOK]
    rolef = role_s[idx].astype(bf16).reshape(TOPK, TOK, CT, 128)
    g_b = np.ascontiguousarray(g.astype(bf16))
    # x pre-transposed per core, two token halves: [2, 128 C-part, CT, TPC/2]
    xb = xf.astype(bf16).reshape(N_CORES, 2, TPC // 2, CT, 128)

    nc = _get_nc()
    in_maps = []
    for i in range(N_CORES):
        sl = slice(i * TPC, (i + 1) * TPC)
        in_maps.append({
            "xs": np.ascontiguousarray(xb[i].transpose(0, 3, 2, 1)),
            "w1t": w1tp, "w3t": w3tp, "w2t": w2tp,
            "rolef": np.ascontiguousarray(
                rolef[:, sl].transpose(0, 3, 2, 1)),
            "g4": np.ascontiguousarray(
                np.broadcast_to(g_b[:, None, sl], (4, 128, TPC))),
        })
    res = run_bass_kernel_spmd(nc, in_maps, core_ids=list(range(N_CORES)),
                               trace=_trace)
    _CACHE["last_results"] = res
    # per-core out is [CT, 128, TPC]; reassemble to [TPC, C]
    out = np.concatenate(
        [np.asarray(r["out"]).transpose(2, 0, 1).reshape(TPC, C)
         for r in res.results], axis=0)
    return out.reshape(B, T, C)


# revision 53
# speedup vs baseline: 1.1993x; 1.1993x over previous
"""Trainium2 Bass kernel for nn_MixtureOfAgents.

Contract: kernel(**inputs) takes FULL unsharded inputs (numpy) and returns the
FULL output [4, 4096, 768] float32.

Strategy (v2):
  - Reference quirk: for each of TOP_K=2 steps, ONE scalar agent id
    (top_i[0, -1, k]) selects the expert pair used for ALL tokens.  The host
    computes the full per-token routing (agent softmax, top-2 renorm, expert
    pair softmax -> 4 gate rows g, plus agent one-hots) and slices the 4
    selected expert FFN blocks.  The device runs only the dense pipeline:
    transpose x, 4x (mm1/mm3 -> silu*mul -> mm2 -> combine), transpose out.
  - Data-parallel over tokens: 8 cores x 2048 tokens, weights replicated.
  - All matmul operands in bf16 (weights/x/hidden); PSUM accumulation fp32;
    combine arithmetic fp32.  Weights are pre-laid on host so each SBUF tile
    is one contiguous DMA.
  - mm1/mm3 stream two 512-token halves per stationary weight load.
"""

import numpy as np

# ---- problem constants (hardcoded; kernel.py must be self-contained) ----
N_CORES = 8
B, T, C = 4, 4096, 768
TOK = B * T              # 16384
TPC = TOK // N_CORES     # 2048 tokens per core
CT = C // 128            # 6 c-tiles
FFN = 2048
FT = FFN // 128          # 16 f-tiles per expert
A = 10                   # n_agents
EPA = 2                  # experts per agent
TOPK = 2
NBLK = TPC // 128        # 16 token-blocks
NTQ = TPC // 512         # 4 token-quarters

_CACHE = {}


def _build_module():
    import concourse.bass as bass
    import concourse.bacc as bacc
    import concourse.mybir as mybir
    import concourse.tile as tile
    from concourse.masks import make_identity
    from contextlib import ExitStack

    f32 = mybir.dt.float32
    bf16 = mybir.dt.bfloat16
    AF = mybir.ActivationFunctionType
    OP = mybir.AluOpType

    nc = bacc.Bacc(target_bir_lowering=False)
    xs = nc.dram_tensor("xs", [2, 128, CT, TPC // 2], bf16, kind="ExternalInput")
    w1t = nc.dram_tensor("w1t", [TOPK, EPA, FT, 128, CT, 128], bf16,
                         kind="ExternalInput")
    w3t = nc.dram_tensor("w3t", [TOPK, EPA, FT, 128, CT, 128], bf16,
                         kind="ExternalInput")
    w2t = nc.dram_tensor("w2t", [TOPK, EPA, CT, 128, FT, 128], bf16,
                         kind="ExternalInput")
    rolef = nc.dram_tensor("rolef", [TOPK, 128, CT, TPC], bf16,
                           kind="ExternalInput")
    g4 = nc.dram_tensor("g4", [TOPK * EPA, 128, TPC], bf16, kind="ExternalInput")
    out = nc.dram_tensor("out", [CT, 128, TPC], f32, kind="ExternalOutput")
    import os as _os
    _dbg = _os.environ.get("KERNEL_DEBUG_DUMPS") == "1"
    if _dbg:
        dbg_hid = nc.dram_tensor("dbg_hid", [128, FT, TPC], bf16,
                                 kind="ExternalOutput")
        dbg_acc = nc.dram_tensor("dbg_acc", [128, CT, TPC], f32,
                                 kind="ExternalOutput")

    with ExitStack() as ctx:
        tc = ctx.enter_context(tile.TileContext(nc))
        const = ctx.enter_context(tc.tile_pool(name="const", bufs=1))
        persist = ctx.enter_context(tc.tile_pool(name="persist", bufs=1))
        w13p = ctx.enter_context(tc.tile_pool(name="w13p", bufs=6))
        w2p = ctx.enter_context(tc.tile_pool(name="w2p", bufs=2))
        btp = ctx.enter_context(tc.tile_pool(name="btp", bufs=2))
        rp = ctx.enter_context(tc.tile_pool(name="rp", bufs=1))
        tmpp = ctx.enter_context(tc.tile_pool(name="tmpp", bufs=2))
        psH = ctx.enter_context(tc.tile_pool(name="psH", bufs=4, space="PSUM"))
        psY = ctx.enter_context(tc.tile_pool(name="psY", bufs=4, space="PSUM"))


        xT = persist.tile([128, CT, TPC], bf16, tag="xT", name="xT")
        hid = persist.tile([128, FT, TPC], bf16, tag="hid", name="hid")
        acc = persist.tile([128, CT, TPC], f32, tag="acc", name="acc")

        # ---- prologue: first-pass weight tiles go out ahead of the bulk
        # x DMA so the PE can start as soon as the first x half lands ----
        pre_w = {}
        for f in range(2):
            w1f = w13p.tile([128, CT, 128], bf16, tag="w13", name=f"w1f_0_0_{f}")
            nc.sync.dma_start(out=w1f, in_=w1t[0, 0, f, :, :, :])
            w3f = w13p.tile([128, CT, 128], bf16, tag="w13", name=f"w3f_0_0_{f}")
            nc.sync.dma_start(out=w3f, in_=w3t[0, 0, f, :, :, :])
            pre_w[f] = (w1f, w3f)
        # x pre-transposed [C-part, c-tile, tok], two contiguous halves
        for h in range(2):
            hs = slice(h * (TPC // 2), (h + 1) * (TPC // 2))
            nc.sync.dma_start(out=xT[:, :, hs], in_=xs[h, :, :, :])

        # ---- main: 4 expert passes ----
        for k in range(TOPK):
            rk = rp.tile([128, CT, TPC], bf16, tag="rk", name=f"rk_{k}")
            for e in range(EPA):
                # mm1 + mm3 + silu*mul -> hid [FFN, TPC]
                for f in range(FT):
                    if k == 0 and e == 0 and f in pre_w:
                        w1f, w3f = pre_w[f]
                    else:
                        w1f = w13p.tile([128, CT, 128], bf16, tag="w13",
                                        name=f"w1f_{k}_{e}_{f}")
                        nc.sync.dma_start(out=w1f, in_=w1t[k, e, f, :, :, :])
                        w3f = w13p.tile([128, CT, 128], bf16, tag="w13",
                                        name=f"w3f_{k}_{e}_{f}")
                        nc.sync.dma_start(out=w3f, in_=w3t[k, e, f, :, :, :])
                    for half in range(2):
                        t0 = slice(half * 1024, half * 1024 + 512)
                        t1 = slice(half * 1024 + 512, half * 1024 + 1024)
                        ph1 = [psH.tile([128, 512], f32, tag="ps_h",
                                        name=f"ph1_{k}_{e}_{f}_{half}_{q}")
                               for q in range(2)]
                        ph3 = [psH.tile([128, 512], f32, tag="ps_h",
                                        name=f"ph3_{k}_{e}_{f}_{half}_{q}")
                               for q in range(2)]
                        # pair token-halves per stationary weight tile
                        for c in range(CT):
                            nc.tensor.matmul(ph1[0], w1f[:, c, :], xT[:, c, t0],
                                             start=(c == 0), stop=(c == CT - 1))
                            nc.tensor.matmul(ph1[1], w1f[:, c, :], xT[:, c, t1],
                                             start=(c == 0), stop=(c == CT - 1))
                        for c in range(CT):
                            nc.tensor.matmul(ph3[0], w3f[:, c, :], xT[:, c, t0],
                                             start=(c == 0), stop=(c == CT - 1))
                            nc.tensor.matmul(ph3[1], w3f[:, c, :], xT[:, c, t1],
                                             start=(c == 0), stop=(c == CT - 1))
                        for q, ts in ((0, t0), (1, t1)):
                            nc.scalar.activation(hid[:, f, ts], ph1[q], AF.Silu)
                            nc.vector.tensor_tensor(hid[:, f, ts], hid[:, f, ts],
                                                    ph3[q], op=OP.mult)

                if _dbg and k == 0 and e == 0:
                    nc.sync.dma_start(out=dbg_hid[:, :, :], in_=hid[:, :, :])
                # gates needed only from the combine stage on
                bt = btp.tile([128, TPC], bf16, tag="bt", name=f"bt_{k}_{e}")
                nc.sync.dma_start(out=bt, in_=g4[2 * k + e, :, :])
                # mm2 + combine into acc
                if e == 0:
                    nc.sync.dma_start(out=rk[:, 0, :], in_=rolef[k, :, 0, :])
                for c in range(CT):
                    if e == 0 and c + 1 < CT:
                        nc.sync.dma_start(out=rk[:, c + 1, :],
                                          in_=rolef[k, :, c + 1, :])
                    w2f = w2p.tile([128, FT, 128], bf16, tag="w2",
                                   name=f"w2f_{k}_{e}_{c}")
                    nc.sync.dma_start(out=w2f, in_=w2t[k, e, c, :, :, :])
                    for tq in range(NTQ):
                        ts = slice(tq * 512, (tq + 1) * 512)
                        py = psY.tile([128, 512], f32, tag="ps_y",
                                      name=f"py_{k}_{e}_{c}_{tq}")
                        for f in range(FT):
                            nc.tensor.matmul(py, w2f[:, f, :], hid[:, f, ts],
                                             start=(f == 0), stop=(f == FT - 1))
                        t1_ = tmpp.tile([128, 512], f32, tag="t1",
                                        name=f"t1_{k}_{e}_{c}_{tq}")
                        nc.vector.tensor_tensor(t1_, bt[:, ts], py,
                                                op=OP.mult)
                        if k == 0 and e == 0:
                            nc.vector.scalar_tensor_tensor(
                                out=acc[:, c, ts], in0=rk[:, c, ts], scalar=1.0,
                                in1=t1_, op0=OP.add, op1=OP.mult)
                        else:
                            nc.vector.scalar_tensor_tensor(
                                out=t1_, in0=rk[:, c, ts], scalar=1.0,
                                in1=t1_, op0=OP.add, op1=OP.mult)
                            nc.vector.tensor_tensor(acc[:, c, ts], acc[:, c, ts],
                                                    t1_, op=OP.add)

                if _dbg and k == 0 and e == 0:
                    nc.sync.dma_start(out=dbg_acc[:, :, :], in_=acc[:, :, :])

        # ---- epilogue: store acc in [c-tile, C-part, tok] layout ----
        for c in range(CT):
            for h in range(2):
                hs = slice(h * (TPC // 2), (h + 1) * (TPC // 2))
                nc.sync.dma_start(out=out[c, :, hs], in_=acc[:, c, hs])

    nc.compile()
    return nc


def _get_nc():
    if "nc" not in _CACHE:
        _CACHE["nc"] = _build_module()
    return _CACHE["nc"]


def _enable_jax_compile_cache():
    try:
        import jax
        jax.config.update("jax_compilation_cache_dir", "/tmp/jax_kernel_cache")
        jax.config.update("jax_persistent_cache_min_compile_time_secs", 1.0)
    except Exception:
        pass


def _host_routing(xf, agent_gate_w, expert_gate_w):
    """Per-token gates exactly as the reference computes them (fp32)."""
    al = xf @ agent_gate_w.T                                    # [TOK, A]
    al = al - al.max(axis=1, keepdims=True)
    aw = np.exp(al)
    aw /= aw.sum(axis=1, keepdims=True)
    order = np.argsort(-aw, axis=1, kind="stable")              # [TOK, A]
    i_k = order[:, :TOPK]                                       # [TOK, 2]
    w_k = np.take_along_axis(aw, i_k, axis=1)                   # [TOK, 2]
    tw = w_k / (w_k.sum(axis=1, keepdims=True) + 1e-6)          # [TOK, 2]

    # scalar agent ids from token (b=0, t=T-1) -> flat row T-1
    sel = [int(i_k[T - 1, k]) * EPA for k in range(TOPK)]

    cols = [sel[0], sel[0] + 1, sel[1], sel[1] + 1]
    el = xf @ expert_gate_w[cols].T                             # [TOK, 4]
    g = np.empty((4, TOK), dtype=np.float32)
    for k in range(TOPK):
        pair = el[:, 2 * k:2 * k + 2]
        pair = pair - pair.max(axis=1, keepdims=True)
        ew = np.exp(pair)
        ew /= ew.sum(axis=1, keepdims=True)
        g[2 * k] = tw[:, k] * ew[:, 0]
        g[2 * k + 1] = tw[:, k] * ew[:, 1]

    onehot = np.zeros((TOPK, A, TOK), dtype=np.float32)
    for k in range(TOPK):
        onehot[k, i_k[:, k], np.arange(TOK)] = 1.0
    return sel, g, onehot


def kernel(x, agent_gate_w, expert_gate_w, role_emb, w1, w2, w3,
           _trace=False, _dtype="f32r"):
    import ml_dtypes
    from concourse.bass_utils import run_bass_kernel_spmd

    _enable_jax_compile_cache()
    bf16 = ml_dtypes.bfloat16

    x = np.asarray(x, dtype=np.float32)
    agent_gate_w = np.asarray(agent_gate_w, dtype=np.float32)
    expert_gate_w = np.asarray(expert_gate_w, dtype=np.float32)
    role_emb = np.asarray(role_emb, dtype=np.float32)
    w1 = np.asarray(w1, dtype=np.float32)
    w2 = np.asarray(w2, dtype=np.float32)
    w3 = np.asarray(w3, dtype=np.float32)

    xf = np.ascontiguousarray(x.reshape(TOK, C))
    sel, g, onehot = _host_routing(xf, agent_gate_w, expert_gate_w)

    rows = [sel[0], sel[0] + 1, sel[1], sel[1] + 1]
    # w1/w3 tiles: [ke, f, p, c, j] with value w[e, f*128+j, c*128+p]
    w1sel = w1[rows].reshape(4, FT, 128, CT, 128).transpose(0, 1, 4, 3, 2)
    w3sel = w3[rows].reshape(4, FT, 128, CT, 128).transpose(0, 1, 4, 3, 2)
    # w2 tiles: [ke, c, p, f, j] with value w2[e, c*128+j, f*128+p]
    w2sel = w2[rows].reshape(4, CT, 128, FT, 128).transpose(0, 1, 4, 3, 2)

    w1tp = np.ascontiguousarray(
        w1sel.reshape(TOPK, EPA, FT, 128, CT, 128).astype(bf16))
    w3tp = np.ascontiguousarray(
        w3sel.reshape(TOPK, EPA, FT, 128, CT, 128).astype(bf16))
    w2tp = np.ascontiguousarray(
        w2sel.reshape(TOPK, EPA, CT, 128, FT, 128).astype(bf16))
    # gathered role factor 0.1*role_emb[idx_k] per token, [TOPK, TOK, CT, 128]
    role_s = 0.1 * role_emb
    idx = np.argmax(onehot, axis=1)                       # [TOPK, TOK]
    rolef = role_s[idx].astype(bf16).reshape(TOPK, TOK, CT, 128)
    g_b = np.ascontiguousarray(g.astype(bf16))
    # x pre-transposed per core, two token halves: [2, 128 C-part, CT, TPC/2]
    xb = xf.astype(bf16).reshape(N_CORES, 2, TPC // 2, CT, 128)

    nc = _get_nc()
    in_maps = []
    for i in range(N_CORES):
        sl = slice(i * TPC, (i + 1) * TPC)
        in_maps.append({
            "xs": np.ascontiguousarray(xb[i].transpose(0, 3, 2, 1)),
            "w1t": w1tp, "w3t": w3tp, "w2t": w2tp,
            "rolef": np.ascontiguousarray(
                rolef[:, sl].transpose(0, 3, 2, 1)),
            "g4": np.ascontiguousarray(
                np.broadcast_to(g_b[:, None, sl], (4, 128, TPC))),
        })
    res = run_bass_kernel_spmd(nc, in_maps, core_ids=list(range(N_CORES)),
                               trace=_trace)
    _CACHE["last_results"] = res
    # per-core out is [CT, 128, TPC]; reassemble to [TPC, C]
    out = np.concatenate(
        [np.asarray(r["out"]).transpose(2, 0, 1).reshape(TPC, C)
         for r in res.results], axis=0)
    return out.reshape(B, T, C)


# revision 59
# speedup vs baseline: 1.2024x; 1.0026x over previous
"""Trainium2 Bass kernel for nn_MixtureOfAgents.

Contract: kernel(**inputs) takes FULL unsharded inputs (numpy) and returns the
FULL output [4, 4096, 768] float32.

Strategy (v2):
  - Reference quirk: for each of TOP_K=2 steps, ONE scalar agent id
    (top_i[0, -1, k]) selects the expert pair used for ALL tokens.  The host
    computes the full per-token routing (agent softmax, top-2 renorm, expert
    pair softmax -> 4 gate rows g, plus agent one-hots) and slices the 4
    selected expert FFN blocks.  The device runs only the dense pipeline:
    transpose x, 4x (mm1/mm3 -> silu*mul -> mm2 -> combine), transpose out.
  - Data-parallel over tokens: 8 cores x 2048 tokens, weights replicated.
  - All matmul operands in bf16 (weights/x/hidden); PSUM accumulation fp32;
    combine arithmetic fp32.  Weights are pre-laid on host so each SBUF tile
    is one contiguous DMA.
  - mm1/mm3 stream two 512-token halves per stationary weight load.
"""

import numpy as np

# ---- problem constants (hardcoded; kernel.py must be self-contained) ----
N_CORES = 8
B, T, C = 4, 4096, 768
TOK = B * T              # 16384
TPC = TOK // N_CORES     # 2048 tokens per core
CT = C // 128            # 6 c-tiles
FFN = 2048
FT = FFN // 128          # 16 f-tiles per expert
A = 10                   # n_agents
EPA = 2                  # experts per agent
TOPK = 2
NBLK = TPC // 128        # 16 token-blocks
NTQ = TPC // 512         # 4 token-quarters

_CACHE = {}


def _build_module():
    import concourse.bass as bass
    import concourse.bacc as bacc
    import concourse.mybir as mybir
    import concourse.tile as tile
    from concourse.masks import make_identity
    from contextlib import ExitStack

    f32 = mybir.dt.float32
    bf16 = mybir.dt.bfloat16
    AF = mybir.ActivationFunctionType
    OP = mybir.AluOpType

    nc = bacc.Bacc(target_bir_lowering=False)
    xs = nc.dram_tensor("xs", [NTQ, 128, CT, 512], bf16, kind="ExternalInput")
    w1t = nc.dram_tensor("w1t", [TOPK, EPA, FT, 128, CT, 128], bf16,
                         kind="ExternalInput")
    w3t = nc.dram_tensor("w3t", [TOPK, EPA, FT, 128, CT, 128], bf16,
                         kind="ExternalInput")
    w2t = nc.dram_tensor("w2t", [TOPK, EPA, CT, 128, FT, 128], bf16,
                         kind="ExternalInput")
    rolef = nc.dram_tensor("rolef", [TOPK, 128, CT, TPC], bf16,
                           kind="ExternalInput")
    g4 = nc.dram_tensor("g4", [TOPK * EPA, 128, TPC], bf16, kind="ExternalInput")
    out = nc.dram_tensor("out", [CT, 128, TPC], f32, kind="ExternalOutput")
    import os as _os
    _dbg = _os.environ.get("KERNEL_DEBUG_DUMPS") == "1"
    if _dbg:
        dbg_hid = nc.dram_tensor("dbg_hid", [128, FT, TPC], bf16,
                                 kind="ExternalOutput")
        dbg_acc = nc.dram_tensor("dbg_acc", [128, CT, TPC], f32,
                                 kind="ExternalOutput")

    with ExitStack() as ctx:
        tc = ctx.enter_context(tile.TileContext(nc))
        const = ctx.enter_context(tc.tile_pool(name="const", bufs=1))
        persist = ctx.enter_context(tc.tile_pool(name="persist", bufs=1))
        w13p = ctx.enter_context(tc.tile_pool(name="w13p", bufs=6))
        w2p = ctx.enter_context(tc.tile_pool(name="w2p", bufs=2))
        btp = ctx.enter_context(tc.tile_pool(name="btp", bufs=2))
        rp = ctx.enter_context(tc.tile_pool(name="rp", bufs=1))
        tmpp = ctx.enter_context(tc.tile_pool(name="tmpp", bufs=2))
        psH = ctx.enter_context(tc.tile_pool(name="psH", bufs=4, space="PSUM"))
        psY = ctx.enter_context(tc.tile_pool(name="psY", bufs=4, space="PSUM"))


        xT = persist.tile([128, NTQ, CT, 512], bf16, tag="xT", name="xT")
        hid = persist.tile([128, FT, TPC], bf16, tag="hid", name="hid")
        acc = persist.tile([128, CT, TPC], f32, tag="acc", name="acc")

        # ---- prologue: first-pass weight tiles go out ahead of the bulk
        # x DMA so the PE can start as soon as the first x half lands ----
        pre_w = {}
        for f in range(2):
            w1f = w13p.tile([128, CT, 128], bf16, tag="w13", name=f"w1f_0_0_{f}")
            nc.sync.dma_start(out=w1f, in_=w1t[0, 0, f, :, :, :])
            w3f = w13p.tile([128, CT, 128], bf16, tag="w13", name=f"w3f_0_0_{f}")
            nc.sync.dma_start(out=w3f, in_=w3t[0, 0, f, :, :, :])
            pre_w[f] = (w1f, w3f)
        # x pre-transposed [C-part, quarter, c-tile, 512]: each quarter is
        # one fully contiguous DMA so the PE starts after the first lands
        for q in range(NTQ):
            nc.sync.dma_start(out=xT[:, q, :, :], in_=xs[q, :, :, :])

        # ---- main: 4 expert passes ----
        for k in range(TOPK):
            rk = rp.tile([128, CT, TPC], bf16, tag="rk", name=f"rk_{k}")
            for e in range(EPA):
                # mm1 + mm3 + silu*mul -> hid [FFN, TPC]
                for f in range(FT):
                    if k == 0 and e == 0 and f in pre_w:
                        w1f, w3f = pre_w[f]
                    else:
                        w1f = w13p.tile([128, CT, 128], bf16, tag="w13",
                                        name=f"w1f_{k}_{e}_{f}")
                        nc.sync.dma_start(out=w1f, in_=w1t[k, e, f, :, :, :])
                        w3f = w13p.tile([128, CT, 128], bf16, tag="w13",
                                        name=f"w3f_{k}_{e}_{f}")
                        nc.sync.dma_start(out=w3f, in_=w3t[k, e, f, :, :, :])
                    for half in range(2):
                        qa, qb = 2 * half, 2 * half + 1
                        t0 = slice(qa * 512, qa * 512 + 512)
                        t1 = slice(qb * 512, qb * 512 + 512)
                        ph1 = [psH.tile([128, 512], f32, tag="ps_h",
                                        name=f"ph1_{k}_{e}_{f}_{half}_{q}")
                               for q in range(2)]
                        ph3 = [psH.tile([128, 512], f32, tag="ps_h",
                                        name=f"ph3_{k}_{e}_{f}_{half}_{q}")
                               for q in range(2)]
                        # pair token-quarters per stationary weight tile
                        for c in range(CT):
                            nc.tensor.matmul(ph1[0], w1f[:, c, :],
                                             xT[:, qa, c, :],
                                             start=(c == 0), stop=(c == CT - 1))
                            nc.tensor.matmul(ph1[1], w1f[:, c, :],
                                             xT[:, qb, c, :],
                                             start=(c == 0), stop=(c == CT - 1))
                        for c in range(CT):
                            nc.tensor.matmul(ph3[0], w3f[:, c, :],
                                             xT[:, qa, c, :],
                                             start=(c == 0), stop=(c == CT - 1))
                            nc.tensor.matmul(ph3[1], w3f[:, c, :],
                                             xT[:, qb, c, :],
                                             start=(c == 0), stop=(c == CT - 1))
                        for q, ts in ((0, t0), (1, t1)):
                            nc.scalar.activation(hid[:, f, ts], ph1[q], AF.Silu)
                            nc.vector.tensor_tensor(hid[:, f, ts], hid[:, f, ts],
                                                    ph3[q], op=OP.mult)

                if _dbg and k == 0 and e == 0:
                    nc.sync.dma_start(out=dbg_hid[:, :, :], in_=hid[:, :, :])
                # gates needed only from the combine stage on
                bt = btp.tile([128, TPC], bf16, tag="bt", name=f"bt_{k}_{e}")
                nc.sync.dma_start(out=bt, in_=g4[2 * k + e, :, :])
                # mm2 + combine into acc
                if e == 0:
                    nc.sync.dma_start(out=rk[:, 0, :], in_=rolef[k, :, 0, :])
                for c in range(CT):
                    if e == 0 and c + 1 < CT:
                        nc.sync.dma_start(out=rk[:, c + 1, :],
                                          in_=rolef[k, :, c + 1, :])
                    w2f = w2p.tile([128, FT, 128], bf16, tag="w2",
                                   name=f"w2f_{k}_{e}_{c}")
                    nc.sync.dma_start(out=w2f, in_=w2t[k, e, c, :, :, :])
                    for tq in range(NTQ):
                        ts = slice(tq * 512, (tq + 1) * 512)
                        py = psY.tile([128, 512], f32, tag="ps_y",
                                      name=f"py_{k}_{e}_{c}_{tq}")
                        for f in range(FT):
                            nc.tensor.matmul(py, w2f[:, f, :], hid[:, f, ts],
                                             start=(f == 0), stop=(f == FT - 1))
                        t1_ = tmpp.tile([128, 512], f32, tag="t1",
                                        name=f"t1_{k}_{e}_{c}_{tq}")
                        nc.vector.tensor_tensor(t1_, bt[:, ts], py,
                                                op=OP.mult)
                        if k == 0 and e == 0:
                            nc.vector.scalar_tensor_tensor(
                                out=acc[:, c, ts], in0=rk[:, c, ts], scalar=1.0,
                                in1=t1_, op0=OP.add, op1=OP.mult)
                        else:
                            nc.vector.scalar_tensor_tensor(
                                out=t1_, in0=rk[:, c, ts], scalar=1.0,
                                in1=t1_, op0=OP.add, op1=OP.mult)
                            nc.vector.tensor_tensor(acc[:, c, ts], acc[:, c, ts],
                                                    t1_, op=OP.add)

                if _dbg and k == 0 and e == 0:
                    nc.sync.dma_start(out=dbg_acc[:, :, :], in_=acc[:, :, :])

        # ---- epilogue: store acc in [c-tile, C-part, tok] layout ----
        for c in range(CT):
            for tq in range(NTQ):
                ts = slice(tq * 512, (tq + 1) * 512)
                nc.sync.dma_start(out=out[c, :, ts], in_=acc[:, c, ts])

    nc.compile()
    return nc


def _get_nc():
    if "nc" not in _CACHE:
        _CACHE["nc"] = _build_module()
    return _CACHE["nc"]


def _enable_jax_compile_cache():
    try:
        import jax
        jax.config.update("jax_compilation_cache_dir", "/tmp/jax_kernel_cache")
        jax.config.update("jax_persistent_cache_min_compile_time_secs", 1.0)
    except Exception:
        pass


def _host_routing(xf, agent_gate_w, expert_gate_w):
    """Per-token gates exactly as the reference computes them (fp32)."""
    al = xf @ agent_gate_w.T                                    # [TOK, A]
    al = al - al.max(axis=1, keepdims=True)
    aw = np.exp(al)
    aw /= aw.sum(axis=1, keepdims=True)
    order = np.argsort(-aw, axis=1, kind="stable")              # [TOK, A]
    i_k = order[:, :TOPK]                                       # [TOK, 2]
    w_k = np.take_along_axis(aw, i_k, axis=1)                   # [TOK, 2]
    tw = w_k / (w_k.sum(axis=1, keepdims=True) + 1e-6)          # [TOK, 2]

    # scalar agent ids from token (b=0, t=T-1) -> flat row T-1
    sel = [int(i_k[T - 1, k]) * EPA for k in range(TOPK)]

    cols = [sel[0], sel[0] + 1, sel[1], sel[1] + 1]
    el = xf @ expert_gate_w[cols].T                             # [TOK, 4]
    g = np.empty((4, TOK), dtype=np.float32)
    for k in range(TOPK):
        pair = el[:, 2 * k:2 * k + 2]
        pair = pair - pair.max(axis=1, keepdims=True)
        ew = np.exp(pair)
        ew /= ew.sum(axis=1, keepdims=True)
        g[2 * k] = tw[:, k] * ew[:, 0]
        g[2 * k + 1] = tw[:, k] * ew[:, 1]

    onehot = np.zeros((TOPK, A, TOK), dtype=np.float32)
    for k in range(TOPK):
        onehot[k, i_k[:, k], np.arange(TOK)] = 1.0
    return sel, g, onehot


def kernel(x, agent_gate_w, expert_gate_w, role_emb, w1, w2, w3,
           _trace=False, _dtype="f32r"):
    import ml_dtypes
    from concourse.bass_utils import run_bass_kernel_spmd

    _enable_jax_compile_cache()
    bf16 = ml_dtypes.bfloat16

    x = np.asarray(x, dtype=np.float32)
    agent_gate_w = np.asarray(agent_gate_w, dtype=np.float32)
    expert_gate_w = np.asarray(expert_gate_w, dtype=np.float32)
    role_emb = np.asarray(role_emb, dtype=np.float32)
    w1 = np.asarray(w1, dtype=np.float32)
    w2 = np.asarray(w2, dtype=np.float32)
    w3 = np.asarray(w3, dtype=np.float32)

    xf = np.ascontiguousarray(x.reshape(TOK, C))
    sel, g, onehot = _host_routing(xf, agent_gate_w, expert_gate_w)

    rows = [sel[0], sel[0] + 1, sel[1], sel[1] + 1]
    # w1/w3 tiles: [ke, f, p, c, j] with value w[e, f*128+j, c*128+p]
    w1sel = w1[rows].reshape(4, FT, 128, CT, 128).transpose(0, 1, 4, 3, 2)
    w3sel = w3[rows].reshape(4, FT, 128, CT, 128).transpose(0, 1, 4, 3, 2)
    # w2 tiles: [ke, c, p, f, j] with value w2[e, c*128+j, f*128+p]
    w2sel = w2[rows].reshape(4, CT, 128, FT, 128).transpose(0, 1, 4, 3, 2)

    w1tp = np.ascontiguousarray(
        w1sel.reshape(TOPK, EPA, FT, 128, CT, 128).astype(bf16))
    w3tp = np.ascontiguousarray(
        w3sel.reshape(TOPK, EPA, FT, 128, CT, 128).astype(bf16))
    w2tp = np.ascontiguousarray(
        w2sel.reshape(TOPK, EPA, CT, 128, FT, 128).astype(bf16))
    # gathered role factor 0.1*role_emb[idx_k] per token, [TOPK, TOK, CT, 128]
    role_s = 0.1 * role_emb
    idx = np.argmax(onehot, axis=1)                       # [TOPK, TOK]
    rolef = role_s[idx].astype(bf16).reshape(TOPK, TOK, CT, 128)
    g_b = np.ascontiguousarray(g.astype(bf16))
    # x pre-transposed per core, token quarters: [NTQ, 128 C-part, CT, 512]
    xb = xf.astype(bf16).reshape(N_CORES, NTQ, 512, CT, 128)

    nc = _get_nc()
    in_maps = []
    for i in range(N_CORES):
        sl = slice(i * TPC, (i + 1) * TPC)
        in_maps.append({
            "xs": np.ascontiguousarray(xb[i].transpose(0, 3, 2, 1)),
            "w1t": w1tp, "w3t": w3tp, "w2t": w2tp,
            "rolef": np.ascontiguousarray(
                rolef[:, sl].transpose(0, 3, 2, 1)),
            "g4": np.ascontiguousarray(
                np.broadcast_to(g_b[:, None, sl], (4, 128, TPC))),
        })
    res = run_bass_kernel_spmd(nc, in_maps, core_ids=list(range(N_CORES)),
                               trace=_trace)
    _CACHE["last_results"] = res
    # per-core out is [CT, 128, TPC]; reassemble to [TPC, C]
    out = np.concatenate(
        [np.asarray(r["out"]).transpose(2, 0, 1).reshape(TPC, C)
         for r in res.results], axis=0)
    return out.reshape(B, T, C)
